# revision 1
# baseline (speedup 1.0000x reference)
"""Batched quantize->matmul->dequantize kernel for 8 Trainium2 NeuronCores.

Problem: input0 [16,1024,1024] f32, input1 [16,1024,1024] f32.
  qa = clip(round(input0*10), -128, 127); qb likewise
  out = (qa @ qb) / 10            # batched, f32

Strategy: shard batch dim across 8 cores (2 batches/core), each core runs an
identical Bass/Tile kernel: quantize both operands to integer-valued bf16
(exact: |q| <= 127 < 256 so bf16 holds the ints exactly; PE multiplies
exactly and accumulates in fp32 PSUM), 128x128x512 matmul tiles with K
accumulation in PSUM, dequant (x0.1) fused into the PSUM->SBUF eviction.

Rounding: round-to-nearest-even exactly like jnp.round via the fp32 magic
constant trick: (10*x + 1.5*2^23) - 1.5*2^23. Valid for |10*x| < 2^22; the
clip to [-128,127] is a no-op for N(0,1) inputs (|10*x| < ~60) and the
matmul would be inexact long before the clip bound matters.

The A operand is laid out [b, K, M] host-side during sharding (the PE's
native stationary-operand layout: matmul computes lhsT.T @ rhs with the
contraction dim on partitions for both operands).
"""

import sys

if "/opt/trn_rl_repo" not in sys.path:
    sys.path.insert(0, "/opt/trn_rl_repo")

import numpy as np

import concourse.bass as bass
import concourse.mybir as mybir
import concourse.tile as tile
from concourse import bacc
from concourse.bass_utils import run_bass_kernel_spmd

N_CORES = 8
B, M, K, N = 16, 1024, 1024, 1024
BPC = B // N_CORES  # batches per core
P = 128
KT = K // P  # k tiles per batch
MT = M // P  # m tiles per batch

DSCALE = 10.0
WSCALE = 10.0
OSCALE = 10.0
MAGIC = 1.5 * 2.0**23  # fp32 round-to-nearest-even constant

f32 = mybir.dt.float32
bf16 = mybir.dt.bfloat16


def _build_kernel(nc: bass.Bass):
    # A arrives pre-arranged [BPC, K, M]; B natural [BPC, K, N].
    a_dram = nc.dram_tensor("input0_t", [BPC, K, M], f32, kind="ExternalInput").ap()
    b_dram = nc.dram_tensor("input1", [BPC, K, N], f32, kind="ExternalInput").ap()
    c_dram = nc.dram_tensor("output", [BPC, M, N], f32, kind="ExternalOutput").ap()

    with tile.TileContext(nc) as tc:
        with (
            tc.tile_pool(name="a_f32", bufs=4) as a_pool,
            tc.tile_pool(name="b_f32", bufs=4) as b_pool,
            tc.tile_pool(name="qa", bufs=BPC * KT) as qa_pool,
            tc.tile_pool(name="qb", bufs=BPC * KT) as qb_pool,
            tc.tile_pool(name="psum", bufs=3, space="PSUM") as psum_pool,
            tc.tile_pool(name="c_f32", bufs=4) as c_pool,
        ):
            for b in range(BPC):
                qa = []
                qb = []
                for k in range(KT):
                    at = a_pool.tile([P, M], f32)
                    nc.sync.dma_start(out=at[:], in_=a_dram[b, k * P : (k + 1) * P, :])
                    # t = 10*x + MAGIC (fp32 RNE add performs the rounding)
                    nc.vector.tensor_scalar(
                        out=at[:],
                        in0=at[:],
                        scalar1=DSCALE,
                        scalar2=MAGIC,
                        op0=mybir.AluOpType.mult,
                        op1=mybir.AluOpType.add,
                    )
                    qat = qa_pool.tile([P, M], bf16)
                    # q = t - MAGIC -> exact small int, cast to bf16 exact
                    nc.gpsimd.tensor_scalar_add(qat[:], at[:], -MAGIC)
                    qa.append(qat)

                    bt = b_pool.tile([P, N], f32)
                    nc.sync.dma_start(out=bt[:], in_=b_dram[b, k * P : (k + 1) * P, :])
                    nc.scalar.activation(
                        bt[:],
                        bt[:],
                        mybir.ActivationFunctionType.Copy,
                        bias=MAGIC,
                        scale=WSCALE,
                    )
                    qbt = qb_pool.tile([P, N], bf16)
                    nc.vector.tensor_scalar_add(qbt[:], bt[:], -MAGIC)
                    qb.append(qbt)

                for m in range(MT):
                    ps = psum_pool.tile([P, N], f32)
                    for k in range(KT):
                        lhsT = qa[k][:, m * P : (m + 1) * P]
                        for nh in range(2):
                            nc.tensor.matmul(
                                ps[:, nh * 512 : (nh + 1) * 512],
                                lhsT,
                                qb[k][:, nh * 512 : (nh + 1) * 512],
                                start=(k == 0),
                                stop=(k == KT - 1),
                            )
                    ct = c_pool.tile([P, N], f32)
                    # dequant fused into the mandatory PSUM->SBUF eviction
                    nc.scalar.activation(
                        ct[:],
                        ps[:],
                        mybir.ActivationFunctionType.Copy,
                        scale=1.0 / OSCALE,
                    )
                    nc.sync.dma_start(
                        out=c_dram[b, m * P : (m + 1) * P, :], in_=ct[:]
                    )


_NC_CACHE = None


def _get_nc():
    global _NC_CACHE
    if _NC_CACHE is None:
        nc = bacc.Bacc("TRN2", target_bir_lowering=False, debug=False,
                       num_devices=N_CORES)
        _build_kernel(nc)
        nc.compile()
        _NC_CACHE = nc
    return _NC_CACHE


def _make_in_maps(input0: np.ndarray, input1: np.ndarray):
    in_maps = []
    for c in range(N_CORES):
        sl = slice(c * BPC, (c + 1) * BPC)
        a_t = np.ascontiguousarray(input0[sl].transpose(0, 2, 1))
        in_maps.append(
            {"input0_t": a_t, "input1": np.ascontiguousarray(input1[sl])}
        )
    return in_maps


def kernel(input0, input1, **run_kwargs):
    input0 = np.asarray(input0, dtype=np.float32)
    input1 = np.asarray(input1, dtype=np.float32)
    assert input0.shape == (B, M, K) and input1.shape == (B, K, N)

    nc = _get_nc()
    res = run_bass_kernel_spmd(
        nc, _make_in_maps(input0, input1), core_ids=list(range(N_CORES)),
        **run_kwargs,
    )
    out = np.concatenate(
        [res.results[c]["output"] for c in range(N_CORES)], axis=0
    )
    if run_kwargs:
        return out, res
    return out


if __name__ == "__main__":
    a = np.random.randn(B, M, K).astype(np.float32)
    bm = np.random.randn(B, K, N).astype(np.float32)
    out = kernel(a, bm)
    print("out", out.shape, out.dtype)


# revision 2
# speedup vs baseline: 2.9697x; 2.9697x over previous
"""Batched quantize->matmul->dequantize kernel for 8 Trainium2 NeuronCores.

Problem: input0 [16,1024,1024] f32, input1 [16,1024,1024] f32.
  qa = clip(round(input0*10), -128, 127); qb likewise
  out = (qa @ qb) / 10            # batched, f32

Strategy: shard the batch dim across 8 cores (2 batches/core); each core runs
an identical Bass/Tile kernel with no communication.

Quantization: one multiply-by-10 with int8 output — the hardware f32->int8
conversion is round-to-nearest-even with saturation, which is exactly
jnp.clip(jnp.round(x*10), -128, 127) (verified on device incl. the
double-rounding and saturation edge cases). The int8 is then cast to bf16
for the PE: ints <= 128 are exact in bf16, products are exact in the PE's
multiply, and the fp32 PSUM accumulation of integer partial sums < 2^24 is
exact, so the matmul result matches the reference bit-for-bit (up to the
final x0.1 vs /10, <= 1 ulp).

Dequant (x0.1) is fused into the mandatory PSUM->SBUF eviction on the
scalar engine.

The A operand is laid out [b, K, M] host-side during sharding (the PE's
native stationary-operand layout: matmul computes lhsT.T @ rhs with the
contraction dim on partitions for both operands).

Engine budget per core (measured per-op costs): DVE ~48 ops (A quant both
steps + B int8->bf16), ACT ~32 ops (B mul->int8 + dequant evictions), PE 256
matmuls of [128k,128m]x[128k,512n], DMA 24 MiB. GPSIMD is left idle on
purpose: its tensor ops run ~15us/tile and its SBUF port lock stalls DVE.
"""

import sys

if "/opt/trn_rl_repo" not in sys.path:
    sys.path.insert(0, "/opt/trn_rl_repo")

import numpy as np

import concourse.bass as bass
import concourse.mybir as mybir
import concourse.tile as tile
from concourse import bacc
from concourse.bass_utils import run_bass_kernel_spmd

N_CORES = 8
B, M, K, N = 16, 1024, 1024, 1024
BPC = B // N_CORES  # batches per core
P = 128
KT = K // P  # k tiles per batch
MT = M // P  # m tiles per batch

DSCALE = 10.0
WSCALE = 10.0
OSCALE = 10.0

f32 = mybir.dt.float32
bf16 = mybir.dt.bfloat16
i8 = mybir.dt.int8


def _build_kernel(nc: bass.Bass):
    # A arrives pre-arranged [BPC, K, M]; B natural [BPC, K, N].
    a_dram = nc.dram_tensor("input0_t", [BPC, K, M], f32, kind="ExternalInput").ap()
    b_dram = nc.dram_tensor("input1", [BPC, K, N], f32, kind="ExternalInput").ap()
    c_dram = nc.dram_tensor("output", [BPC, M, N], f32, kind="ExternalOutput").ap()

    with tile.TileContext(nc) as tc:
        with (
            tc.tile_pool(name="a_f32", bufs=3) as a_pool,
            tc.tile_pool(name="b_f32", bufs=3) as b_pool,
            tc.tile_pool(name="a_i8", bufs=3) as ai_pool,
            tc.tile_pool(name="b_i8", bufs=3) as bi_pool,
            tc.tile_pool(name="qa", bufs=BPC * KT) as qa_pool,
            tc.tile_pool(name="qb", bufs=BPC * KT) as qb_pool,
            tc.tile_pool(name="psum", bufs=4, space="PSUM") as psum_pool,
            tc.tile_pool(name="c_f32", bufs=4) as c_pool,
        ):
            for b in range(BPC):
                qa = []
                qb = []
                for k in range(KT):
                    at = a_pool.tile([P, M], f32)
                    nc.sync.dma_start(out=at[:], in_=a_dram[b, k * P : (k + 1) * P, :])
                    ai = ai_pool.tile([P, M], i8)
                    # f32->int8 convert = RNE + saturate == clip(round(10x))
                    nc.vector.tensor_scalar_mul(ai[:], at[:], DSCALE)
                    qat = qa_pool.tile([P, M], bf16)
                    nc.vector.tensor_copy(out=qat[:], in_=ai[:])
                    qa.append(qat)

                    bt = b_pool.tile([P, N], f32)
                    nc.sync.dma_start(out=bt[:], in_=b_dram[b, k * P : (k + 1) * P, :])
                    bi = bi_pool.tile([P, N], i8)
                    nc.scalar.activation(
                        bi[:],
                        bt[:],
                        mybir.ActivationFunctionType.Copy,
                        scale=WSCALE,
                    )
                    qbt = qb_pool.tile([P, N], bf16)
                    nc.vector.tensor_copy(out=qbt[:], in_=bi[:])
                    qb.append(qbt)

                for m in range(MT):
                    ps = psum_pool.tile([P, N], f32)
                    for k in range(KT):
                        lhsT = qa[k][:, m * P : (m + 1) * P]
                        for nh in range(2):
                            nc.tensor.matmul(
                                ps[:, nh * 512 : (nh + 1) * 512],
                                lhsT,
                                qb[k][:, nh * 512 : (nh + 1) * 512],
                                start=(k == 0),
                                stop=(k == KT - 1),
                            )
                    ct = c_pool.tile([P, N], f32)
                    # dequant fused into the mandatory PSUM->SBUF eviction
                    nc.scalar.activation(
                        ct[:],
                        ps[:],
                        mybir.ActivationFunctionType.Copy,
                        scale=1.0 / OSCALE,
                    )
                    nc.sync.dma_start(
                        out=c_dram[b, m * P : (m + 1) * P, :], in_=ct[:]
                    )


_NC_CACHE = None


def _get_nc():
    global _NC_CACHE
    if _NC_CACHE is None:
        nc = bacc.Bacc("TRN2", target_bir_lowering=False, debug=False,
                       num_devices=N_CORES)
        _build_kernel(nc)
        nc.compile()
        _NC_CACHE = nc
    return _NC_CACHE


def _make_in_maps(input0: np.ndarray, input1: np.ndarray):
    in_maps = []
    for c in range(N_CORES):
        sl = slice(c * BPC, (c + 1) * BPC)
        a_t = np.ascontiguousarray(input0[sl].transpose(0, 2, 1))
        in_maps.append(
            {"input0_t": a_t, "input1": np.ascontiguousarray(input1[sl])}
        )
    return in_maps


def kernel(input0, input1, **run_kwargs):
    input0 = np.asarray(input0, dtype=np.float32)
    input1 = np.asarray(input1, dtype=np.float32)
    assert input0.shape == (B, M, K) and input1.shape == (B, K, N)

    nc = _get_nc()
    res = run_bass_kernel_spmd(
        nc, _make_in_maps(input0, input1), core_ids=list(range(N_CORES)),
        **run_kwargs,
    )
    out = np.concatenate(
        [res.results[c]["output"] for c in range(N_CORES)], axis=0
    )
    if run_kwargs:
        return out, res
    return out


if __name__ == "__main__":
    a = np.random.randn(B, M, K).astype(np.float32)
    bm = np.random.randn(B, K, N).astype(np.float32)
    out = kernel(a, bm)
    print("out", out.shape, out.dtype)
